# revision 6
# baseline (speedup 1.0000x reference)
"""Trainium2 Bass/Tile kernel for ExtAttentionPool (nn_ExtAttentionPool).

Math (per sample b):
    S[i, o]  = sum_d L[i, d] * W[o, d]
    E[o, i]  = exp(S[i,o]/O)            (bias cancels in the softmax over i)
    Z[o]     = sum_i E[o, i]
    OUT[o,t] = (1/Z[o]) * sum_i E[o, i] * L[t, i]
    result row b = OUT flattened (O-major), shape (O*T,)

Sharding: data-parallel over batch B=16 across 8 cores (2 samples/core).

Implementation:
  - logits are cast to bf16 AND transposed/swizzled on the host into
    y[kp, p, s, c, t] with d = 128c+p, t_global = TB*kp + t.  Both matmuls
    contract over logits' d axis, so the device needs Lt (d on partitions);
    doing the transpose host-side removes all on-chip transpose matmuls and
    the PSUM->SBUF copy traffic they require.
  - The per-core load is NKP contiguous 1 MiB DMA chunks (chunk kp = all
    data for t-block kp of both samples), issued up front on the sync
    HWDGE queue; large transfers run near the ~358 GB/s per-core HBM cap.
  - The two samples are packed side by side in PE column groups: sample 0
    writes PSUM partitions 0:10, sample 1 partitions 32:42
    (tile_position=(0,32)).  Packed matmul pairs stream concurrently, so
    mm1+mm2 for both samples cost barely more than for one.
  - mm1 for t-block kp runs as soon as chunk kp lands (contraction data
    for a t-block arrives together).  exp (with Z row-sum accumulated by
    the activation) and the tiny E-transpose (identity matmul) follow, and
    mm2 is an L-shaped (j, m) grid: column m=kp as chunk kp lands, rows
    j=2kp,2kp+1 once ec blocks exist.  Only ~14 matmul pairs + one exp
    remain after the last byte arrives.
  - 1/Z rides the PSUM->SBUF copies at the end (ScalarE/DVE alternating),
    one output DMA per (sample, t-block) on the two HWDGE queues.
"""

import numpy as np
import ml_dtypes
from contextlib import ExitStack

_np_bf16 = ml_dtypes.bfloat16

import concourse.bass as bass
import concourse.mybir as mybir
import concourse.tile as tile
from concourse import bacc
from concourse.bass_utils import run_bass_kernel_spmd
from concourse.masks import make_identity

F32 = mybir.dt.float32
BF16 = mybir.dt.bfloat16

N_CORES = 8
B_FULL = 16
P = 128
T = 1024
D = 1024
O = 10
NKP = 4              # DMA chunks / t-super-blocks per core
TB = T // NKP        # 256: t-columns per chunk
ND = D // P          # 8 contraction steps
NJ = T // P          # 8 E-transpose blocks
SOFF = 32            # partition strip offset per sample (col group)


def build_nc(b_per=2, warmup_mms=9):
    """Per-core Bass program (bf16 compute, both samples packed)."""
    nc = bacc.Bacc(
        "TRN2", target_bir_lowering=False, debug=False, enable_asserts=False
    )
    y = nc.dram_tensor("y", (NKP, P, b_per, ND, TB), BF16, kind="ExternalInput").ap()
    wt_in = nc.dram_tensor("wt", (P, ND, O), BF16, kind="ExternalInput").ap()
    out = nc.dram_tensor("out", (b_per, O * T), F32, kind="ExternalOutput").ap()

    with tile.TileContext(nc) as tc, ExitStack() as ctx:
        singles = ctx.enter_context(tc.tile_pool(name="singles", bufs=1))
        sc_ps = ctx.enter_context(tc.tile_pool(name="sc", bufs=1, space="PSUM"))
        o_ps = ctx.enter_context(tc.tile_pool(name="o", bufs=1, space="PSUM"))
        et_ps = ctx.enter_context(tc.tile_pool(name="et", bufs=2, space="PSUM"))

        # --- the whole load: NKP 1 MiB chunks, in order, on the sync ring
        lt = singles.tile([P, NKP, b_per, ND, TB], BF16)
        for kp in range(NKP):
            nc.sync.dma_start(out=lt[:, kp], in_=y[kp])

        # small inputs ride the other HWDGE ring
        wt_sb = singles.tile([P, ND, O], BF16)
        nc.scalar.dma_start(out=wt_sb, in_=wt_in)

        # scores / mm2-out PSUM: [42, 512] banks, strips per sample
        sc_t = [sc_ps.tile([SOFF + O, 2 * TB], F32, name=f"sc{h}") for h in range(2)]
        # one PSUM bank per output t-block: a start=True matmul clears the
        # has_written bits for its whole (bank x partition-row), so no two
        # concurrently-open accumulation groups may share bank+partitions.
        o_t = [o_ps.tile([P, 2 * TB], F32, name=f"ot{m}") for m in range(NKP)]

        # --- PE warmup: >=3.41us of back-to-back matmuls flips the HAM
        # clock gate to 2.4 GHz while chunk 0 is still streaming in.  A
        # zeroed tile is ready ~1.3us before make_identity, so warm on that
        # and build the identity (needed only by the E-transpose) after.
        wsrc = singles.tile([P, 4 * P], BF16)
        nc.gpsimd.memset(wsrc, 0.0)
        for i in range(warmup_mms):
            nc.tensor.matmul(
                o_t[0][64:P, :], lhsT=wsrc[:, 0:64], rhs=wsrc,
                start=True, stop=True, tile_position=(0, 64),
            )

        ident = singles.tile([P, P], BF16)
        make_identity(nc, ident)

        e_sb = singles.tile([SOFF + O, T], BF16)
        ec = singles.tile([P, b_per, NJ, O], BF16)
        zparts = singles.tile([SOFF + O, NKP], F32)
        o_sb = singles.tile([SOFF + O, T], F32)  # scaled output staging

        def strip(ap2d, s, cols):
            return ap2d[SOFF * s : SOFF * s + O, cols]

        def sc_loc(kp):
            return sc_t[kp // 2], slice((kp % 2) * TB, (kp % 2) * TB + TB)

        def mm2_pair(j, m, s):
            nc.tensor.matmul(
                strip(o_t[m], s, slice(0, TB)),
                lhsT=ec[:, s, j, :],
                rhs=lt[:, m, s, j, :],
                start=(j == 0),
                stop=(j == NJ - 1),
                tile_position=(0, SOFF * s),
            )

        out2d = [out[s].rearrange("(o t) -> o t", o=O) for s in range(b_per)]

        for kp in range(NKP):
            sct, cols = sc_loc(kp)
            # mm1 for t-block kp (both samples packed per contraction step)
            for c in range(ND):
                for s in range(b_per):
                    nc.tensor.matmul(
                        strip(sct, s, cols),
                        lhsT=wt_sb[:, c, :],
                        rhs=lt[:, kp, s, c, :],
                        start=(c == 0),
                        stop=(c == ND - 1),
                        tile_position=(0, SOFF * s),
                    )
            # mm2 column m=kp for all ec blocks already available
            for j in range(2 * kp):
                for s in range(b_per):
                    mm2_pair(j, kp, s)
            # exp with Z row-sum accumulation (one op covers both sample
            # strips; rows 10:32 are garbage and never read)
            ecols = slice(kp * TB, (kp + 1) * TB)
            nc.scalar.activation(
                out=e_sb[:, ecols],
                in_=sct[:, cols],
                func=mybir.ActivationFunctionType.Exp,
                scale=1.0 / O,
                accum_out=zparts[:, kp : kp + 1],
            )
            # E-transpose for the two fresh 128-blocks
            for j in (2 * kp, 2 * kp + 1):
                for s in range(b_per):
                    etp = et_ps.tile([P, O], F32, tag="et", name=f"et{kp}_{j}_{s}")
                    nc.tensor.matmul(
                        etp,
                        lhsT=e_sb[SOFF * s : SOFF * s + O, j * P : (j + 1) * P],
                        rhs=ident[SOFF * s : SOFF * s + O, SOFF * s : SOFF * s + O],
                        start=True, stop=True,
                    )
                    nc.vector.tensor_copy(ec[:, s, j, :], etp)
            # mm2 rows j=2kp, 2kp+1 for columns m<=kp (m-major so early
            # columns stop first and their scale/store can begin)
            for m in range(kp + 1):
                for j in (2 * kp, 2 * kp + 1):
                    for s in range(b_per):
                        mm2_pair(j, m, s)

        # softmax denominator per strip
        zsum = singles.tile([SOFF + O, 1], F32)
        nc.vector.reduce_sum(zsum, zparts, axis=mybir.AxisListType.X)
        rz = singles.tile([SOFF + O, 1], F32)
        nc.vector.reciprocal(rz, zsum)

        # scale by 1/Z on the PSUM->SBUF copy: one [42, TB] op per output
        # t-block (bank), ScalarE on banks 0/1, DVE on banks 2/3 so the two
        # engines never contend for the same PSUM bank.
        for m in range(NKP):
            dcols = slice(m * TB, (m + 1) * TB)
            if m < 2:
                nc.scalar.activation(
                    out=o_sb[0:SOFF + O, dcols], in_=o_t[m][0:SOFF + O, 0:TB],
                    func=mybir.ActivationFunctionType.Copy, scale=rz,
                )
            else:
                nc.vector.tensor_scalar_mul(
                    o_sb[0:SOFF + O, dcols], o_t[m][0:SOFF + O, 0:TB], rz
                )
            if m % 2 == 1:  # halves 0:512 / 512:1024 complete -> store
                hcols = slice((m - 1) * TB, (m + 1) * TB)
                for s in range(b_per):
                    eng = nc.sync if s == 0 else (nc.gpsimd if m == 1 else nc.scalar)
                    eng.dma_start(
                        out=out2d[s][:, hcols], in_=strip(o_sb, s, hcols)
                    )

    nc.compile()
    return nc


_NC = None
TRACE = False
LAST_RESULT = None
BUILD_KWARGS = {}


def _get_nc():
    global _NC
    if _NC is None:
        _NC = build_nc(**BUILD_KWARGS)
    return _NC


def kernel(logits, decision, W, b):
    """Full-input entry point: shards batch over 8 cores, returns (16, 10240)."""
    global LAST_RESULT
    lg = np.asarray(logits, dtype=np.float32).astype(_np_bf16)
    Od, Dd = W.shape
    # wt[p, c, o] = W[o, 128c + p]  (host-side transpose of the tiny weight)
    wt = np.ascontiguousarray(
        np.asarray(W, dtype=np.float32).T
        .reshape(Dd // P, P, Od)
        .transpose(1, 0, 2)
    ).astype(_np_bf16)
    nc = _get_nc()
    bp = B_FULL // N_CORES
    in_maps = []
    for i in range(N_CORES):
        pair = lg[i * bp : (i + 1) * bp]  # (2, T, D)
        # y[kp, p, s, c, t] = pair[s, TB*kp + t, 128c + p]
        yarr = np.ascontiguousarray(
            pair.reshape(bp, NKP, TB, ND, P).transpose(1, 4, 0, 3, 2)
        )
        in_maps.append({"y": yarr, "wt": wt})
    res = run_bass_kernel_spmd(nc, in_maps, core_ids=list(range(N_CORES)), trace=TRACE)
    LAST_RESULT = res
    return np.concatenate([res.results[i]["out"] for i in range(N_CORES)], axis=0)
